# revision 16
# baseline (speedup 1.0000x reference)
"""Trainium2 Bass kernel for ConditionalLatentTrajectoryGenerator.

2-layer GRU rollout (B=128, T=512, H=1024, L=C=256) with FiLM conditioning
and an autoregressive linear head.

Sharding: data-parallel, batch 16 per core across 8 cores (weights replicated).

Per-core mapping: batch (16) is the stationary operand of every matmul
(lhsT = x.T [K,16]); weights are the moving operand, pre-permuted into 4
column-groups (tile_position col-tiling) so four weight streams run
concurrently on the PE array. Weights live in SBUF in bf16. PSUM accumulates
gi+gh for the r/z gates; per-example constants (cond-emb contribution,
biases, FiLM beta folded through the head) are added with a K=16 identity
matmul.

State h is kept striped (group g at partitions 32g..32g+16, hidden slice
[256g, 256g+256)). The x.T stationaries are refreshed each step with the
DVE 32x32 block transpose (SBUF->SBUF): out[32g+j, 32c+b] = h[b, 256g+32c+j].
The resulting block-scrambled hidden order is absorbed into the host-side
weight row permutation (moving row p of K-chunk c is hidden
S*(p//32) + 32c + p%32, S = per-stripe hidden span).
"""

import os
import sys
import numpy as np

sys.path.insert(0, "/opt/trn_rl_repo")

import ml_dtypes  # noqa: E402
from concourse import bass, bacc, mybir, tile  # noqa: E402
from concourse import bass_utils  # noqa: E402

F32 = mybir.dt.float32
BF16 = mybir.dt.bfloat16
NPBF16 = ml_dtypes.bfloat16

H = 1024
L = 256
C = 256
B = 128
NCORES = 8
BC = B // NCORES  # 16 batch per core
G = 4             # column groups / stripes
HG = H // G       # 256 hidden per group
LG = L // G       # 64 latent cols per group
U = 16            # steps per hardware-loop body
LAST_EXEC_NS = None
LAST_RESULT = None

if os.environ.get("K_LDW_OPT", "0") == "1":
    _orig_run_command = bass_utils.run_command

    def _run_command_ldwopt(cmd, **kw):
        cmd = ["--enable-ldw-opt=true" if c == "--enable-ldw-opt=false" else c
               for c in cmd]
        return _orig_run_command(cmd, **kw)

    bass_utils.run_command = _run_command_ldwopt


def _striped_batch(x):
    """[BC, 4*S] -> [128, S] with stripe g at partitions 32g..32g+BC."""
    S = x.shape[1] // G
    out = np.zeros((128, S), dtype=x.dtype)
    for g in range(G):
        out[32 * g:32 * g + BC, :] = x[:, g * S:(g + 1) * S]
    return out


def _scrambledT(x):
    """[BC, K] -> [128, K//4] block-transposed layout.

    out[32g+j, 32c+b] = x[b, S*g + 32c + j], S = K//4 — matches what
    nc.vector.transpose produces from the striped batch layout.
    """
    K = x.shape[1]
    S = K // 4
    nch = S // 32
    out = np.zeros((128, 32 * nch), dtype=x.dtype)
    for g in range(G):
        for c in range(nch):
            blk = x[:, S * g + 32 * c:S * g + 32 * c + 32]  # [BC, 32]
            out[32 * g:32 * g + 32, 32 * c:32 * c + BC][:blk.shape[1], :] = blk.T
    return out


def _k_index(K):
    """kidx[c, p] = hidden index feeding moving-row p of K-chunk c."""
    S = K // 4
    nch = S // 32
    p = np.arange(128)
    return np.stack([S * (p // 32) + 32 * c + (p % 32) for c in range(nch)])


def _moving_weights(w):
    """w [rows, K] (rows already output-permuted) -> [128, nch*G*ncols] bf16.

    Column (c, g, j) at c*G*ncols + g*ncols + j holds w[g*ncols+j, kidx[c, p]]
    for partition p.
    """
    K = w.shape[1]
    kidx = _k_index(K)                      # [nch, 128]
    ncols = w.shape[0] // G
    sel = w.T[kidx]                         # [nch, 128, G*ncols]
    arr = sel.transpose(1, 0, 2).reshape(128, kidx.shape[0] * G * ncols)
    return np.ascontiguousarray(arr.astype(NPBF16))


def _const_cols(c, perm):
    return np.ascontiguousarray(c[:, perm].astype(NPBF16))


def _build_program(T, emit_hn1, emit_rz2, emit_inn2, emit_hn2,
                   use_hw_loop=True):
    nc = bacc.Bacc("TRN2", target_bir_lowering=False, debug=False,
                   num_devices=NCORES)

    def din(name, shape, dt):
        return nc.dram_tensor(name, list(shape), dt, kind="ExternalInput")

    d_wa_rz = din("wa_rz", [128, 2 * G * 512], BF16)
    d_wa_inn = din("wa_inn", [128, 2 * G * 256], BF16)
    d_wb_rz = din("wb_rz", [128, 8 * G * 512], BF16)
    d_wb_hn = din("wb_hn", [128, 8 * G * 256], BF16)
    d_wc_rz = din("wc_rz", [128, 8 * G * 512], BF16)
    d_wc_inn = din("wc_inn", [128, 8 * G * 256], BF16)
    d_wd_rz = din("wd_rz", [128, 8 * G * 512], BF16)
    d_wd_hn = din("wd_hn", [128, 8 * G * 256], BF16)
    d_wh = din("wh", [128, 8 * G * LG], BF16)
    d_crz1 = din("crz1", [BC, G * 512], BF16)
    d_cinn1 = din("cinn1", [BC, G * 256], BF16)
    d_cz = din("cz", [BC, G * LG], BF16)
    d_chn1 = din("chn1", [BC, G * 256], BF16) if emit_hn1 else None
    d_crz2 = din("crz2", [BC, G * 512], BF16) if emit_rz2 else None
    d_cinn2 = din("cinn2", [BC, G * 256], BF16) if emit_inn2 else None
    d_chn2 = din("chn2", [BC, G * 256], BF16) if emit_hn2 else None
    d_scaleT = din("scaleT", [128, 256], BF16)
    d_ident = din("ident", [BC, BC], BF16)
    d_h1s = din("h1s0", [128, HG], BF16)
    d_h2s = din("h2s0", [128, HG], BF16)
    d_h1T = din("h1T0", [128, 256], BF16)
    d_h2T = din("h2T0", [128, 256], BF16)
    d_zT = din("zT0", [128, 64], BF16)

    d_out = nc.dram_tensor("out", [128, T * LG], F32, kind="ExternalOutput")

    def sb(name, shape, dt):
        return nc.alloc_sbuf_tensor(name, list(shape), dt)

    s_wa_rz = sb("s_wa_rz", [128, 2 * G * 512], BF16)
    s_wa_inn = sb("s_wa_inn", [128, 2 * G * 256], BF16)
    s_wb_rz = sb("s_wb_rz", [128, 8 * G * 512], BF16)
    s_wb_hn = sb("s_wb_hn", [128, 8 * G * 256], BF16)
    s_wc_rz = sb("s_wc_rz", [128, 8 * G * 512], BF16)
    s_wc_inn = sb("s_wc_inn", [128, 8 * G * 256], BF16)
    s_wd_rz = sb("s_wd_rz", [128, 8 * G * 512], BF16)
    s_wd_hn = sb("s_wd_hn", [128, 8 * G * 256], BF16)
    s_wh = sb("s_wh", [128, 8 * G * LG], BF16)
    s_crz1 = sb("s_crz1", [BC, G * 512], BF16)
    s_cinn1 = sb("s_cinn1", [BC, G * 256], BF16)
    s_cz = sb("s_cz", [BC, G * LG], BF16)
    s_chn1 = sb("s_chn1", [BC, G * 256], BF16) if emit_hn1 else None
    s_crz2 = sb("s_crz2", [BC, G * 512], BF16) if emit_rz2 else None
    s_cinn2 = sb("s_cinn2", [BC, G * 256], BF16) if emit_inn2 else None
    s_chn2 = sb("s_chn2", [BC, G * 256], BF16) if emit_hn2 else None
    s_scaleT = sb("s_scaleT", [128, 256], BF16)
    s_ident = sb("s_ident", [BC, BC], BF16)
    s_h1s = [sb(f"s_h1s{i}", [128, HG], BF16) for i in range(2)]
    s_h2s = [sb(f"s_h2s{i}", [128, HG], BF16) for i in range(2)]
    s_h1T = [sb(f"s_h1T{i}", [128, 256], BF16) for i in range(2)]
    s_h2T = [sb(f"s_h2T{i}", [128, 256], BF16) for i in range(2)]
    s_zT = [sb(f"s_zT{i}", [128, 64], BF16) for i in range(2)]
    s_ring = sb("s_ring", [128, U * LG], F32)

    with tile.TileContext(nc) as tc:
        loads = [
            (s_wa_rz, d_wa_rz), (s_wa_inn, d_wa_inn), (s_wb_rz, d_wb_rz),
            (s_wb_hn, d_wb_hn), (s_wc_rz, d_wc_rz), (s_wc_inn, d_wc_inn),
            (s_wd_rz, d_wd_rz), (s_wd_hn, d_wd_hn), (s_wh, d_wh),
            (s_crz1, d_crz1), (s_cinn1, d_cinn1), (s_cz, d_cz),
            (s_scaleT, d_scaleT), (s_ident, d_ident),
            (s_h1s[0], d_h1s), (s_h2s[0], d_h2s),
            (s_h1T[0], d_h1T), (s_h2T[0], d_h2T), (s_zT[0], d_zT),
        ]
        for s_opt, d_opt in ((s_chn1, d_chn1), (s_crz2, d_crz2),
                             (s_cinn2, d_cinn2), (s_chn2, d_chn2)):
            if s_opt is not None:
                loads.append((s_opt, d_opt))
        for s_t, d_t in loads:
            nc.sync.dma_start(s_t[:], d_t.ap())

        with tc.tile_pool(name="sp", bufs=2) as sp, \
             tc.tile_pool(name="pp", bufs=1, space="PSUM") as pp:

            P1rz = pp.tile([128, 512], F32, tag="p1rz", name="p1rz")
            P1inn = pp.tile([128, 256], F32, tag="p1inn", name="p1inn")
            P1hn = pp.tile([128, 256], F32, tag="p1hn", name="p1hn")
            P2rz = pp.tile([128, 512], F32, tag="p2rz", name="p2rz")
            P2inn = pp.tile([128, 256], F32, tag="p2inn", name="p2inn")
            P2hn = pp.tile([128, 256], F32, tag="p2hn", name="p2hn")
            Pzs = [pp.tile([128, LG], F32, tag=f"pz{i}", name=f"pz{i}")
                   for i in range(2)]
            # Initialize the never-matmul-written garbage stripes once with a
            # full-partition zero matmul (only Matmult/Memset may write PSUM;
            # DVE memset/copy to PSUM fails walrus ISA checks).
            s_zmm = sp.tile([16, 512], BF16, tag="zmm", name="s_zmm")
            nc.vector.memset(s_zmm[:], 0.0)
            for ptile, w in ((P1rz, 512), (P1inn, 256), (P1hn, 256),
                             (P2rz, 512), (P2inn, 256), (P2hn, 256),
                             (Pzs[0], LG), (Pzs[1], LG)):
                nc.tensor.matmul(ptile[:, 0:w], s_zmm[:, 0:128], s_zmm[:, 0:w],
                                 start=True, stop=True, skip_group_check=True)

            yT_of = {}

            def mm(*a, **kw):
                nc.tensor.matmul(*a, skip_group_check=True, **kw)

            def lT(t, c):
                return t[:, 32 * c:32 * c + BC]

            def emit_a_ident(u):
                """L1 const openers (always ready once the previous step's
                sigmoids have read the P1 banks)."""
                for g in range(G):
                    mm(P1rz[32 * g:32 * g + BC, :], s_ident[:],
                       s_crz1[:, g * 512:g * 512 + 512],
                       start=True, stop=False, tile_position=(0, 32 * g))
                    mm(P1inn[32 * g:32 * g + BC, :], s_ident[:],
                       s_cinn1[:, g * 256:g * 256 + 256],
                       start=True, stop=False, tile_position=(0, 32 * g))

            def emit_a_h(u, ks):
                """L1 h1 recurrent part for k in ks (k<4 only needs the first
                128-col half of h1T)."""
                p = u % 2
                for k in ks:
                    for g in range(G):
                        mm(P1rz[32 * g:32 * g + BC, :], lT(s_h1T[p], k),
                           s_wb_rz[:, (k * G + g) * 512:(k * G + g) * 512 + 512],
                           start=False, stop=False, tile_position=(0, 32 * g))
                        mm(P1hn[32 * g:32 * g + BC, :], lT(s_h1T[p], k),
                           s_wb_hn[:, (k * G + g) * 256:(k * G + g) * 256 + 256],
                           start=(k == 0),
                           stop=(k == 7 and not emit_hn1),
                           tile_position=(0, 32 * g))
                if emit_hn1 and 7 in ks:
                    for g in range(G):
                        mm(P1hn[32 * g:32 * g + BC, :], s_ident[:],
                           s_chn1[:, g * 256:g * 256 + 256],
                           start=False, stop=True, tile_position=(0, 32 * g))

            def emit_tail(u):
                """Head chunks k>=4 (need yT half 1 of step u), the Pz const
                closer, and the z feedback transpose + output-ring copy.
                Emitted inside step u+1's stream (after its L1h phase) so the
                in-order PE queue does not stall on yT half 1."""
                Pz = Pzs[u % 2]
                pw = (u + 1) % 2
                yT = yT_of.pop(u)
                for k in range(4, 8):
                    for g in range(G):
                        mm(Pz[32 * g:32 * g + BC, :], lT(yT, k),
                           s_wh[:, (k * G + g) * LG:(k * G + g) * LG + LG],
                           start=False, stop=False, tile_position=(0, 32 * g))
                for g in range(G):
                    mm(Pz[32 * g:32 * g + BC, :], s_ident[:],
                       s_cz[:, g * LG:g * LG + LG],
                       start=False, stop=True, tile_position=(0, 32 * g))
                zb = sp.tile([128, LG], BF16, tag="zb", name=f"zb_{u}")
                nc.scalar.copy(zb[:], Pz[:])
                nc.vector.transpose(s_zT[pw][:], zb[:])
                nc.vector.tensor_copy(s_ring[:, u * LG:(u + 1) * LG], Pz[:])

            def emit_z(u):
                """L1 gi latent part: closes the P1rz/P1inn groups. Needs
                zT (produced at the very end of the previous step)."""
                p = u % 2
                for k in range(2):
                    for g in range(G):
                        mm(P1rz[32 * g:32 * g + BC, :], lT(s_zT[p], k),
                           s_wa_rz[:, (k * G + g) * 512:(k * G + g) * 512 + 512],
                           start=False, stop=(k == 1), tile_position=(0, 32 * g))
                        mm(P1inn[32 * g:32 * g + BC, :], lT(s_zT[p], k),
                           s_wa_inn[:, (k * G + g) * 256:(k * G + g) * 256 + 256],
                           start=False, stop=(k == 1), tile_position=(0, 32 * g))

            def emit_gh(u, ks):
                """L2 gh part (h2T from previous step). Streams while the L1
                elementwise chain runs, hiding it from the in-order PE.
                k=0..1 are emitted at the end of the previous step's stream
                to cover the Pz->zb->zT serial tail."""
                p = u % 2
                for k in ks:
                    for g in range(G):
                        mm(P2rz[32 * g:32 * g + BC, :], lT(s_h2T[p], k),
                           s_wd_rz[:, (k * G + g) * 512:(k * G + g) * 512 + 512],
                           start=(k == 0), stop=False, tile_position=(0, 32 * g))
                        mm(P2hn[32 * g:32 * g + BC, :], lT(s_h2T[p], k),
                           s_wd_hn[:, (k * G + g) * 256:(k * G + g) * 256 + 256],
                           start=(k == 0),
                           stop=(k == 7 and not emit_hn2),
                           tile_position=(0, 32 * g))
                if emit_hn2 and 7 in ks:
                    for g in range(G):
                        mm(P2hn[32 * g:32 * g + BC, :], s_ident[:],
                           s_chn2[:, g * 256:g * 256 + 256],
                           start=False, stop=True, tile_position=(0, 32 * g))

            def gru_elementwise(u, Prz, Pinn, Phn, h_prev, h_out, h_outT,
                                tagp):
                """Column-split (two 128-col halves) GRU cell update.

                Emits ACT sigmoid/tanh and DVE mul/add/STT per half so the
                two halves pipeline across the two engines, and each half's
                32x32 block transpose lands as soon as that half of h_out is
                done (feeding the k<4 / k>=4 moving chunks)."""
                r = sp.tile([128, 256], BF16, tag=f"r{tagp}", name=f"r{tagp}_{u}")
                zz = sp.tile([128, 256], BF16, tag=f"z{tagp}", name=f"z{tagp}_{u}")
                t1 = sp.tile([128, 256], BF16, tag=f"t1{tagp}", name=f"t1{tagp}_{u}")
                t2 = sp.tile([128, 256], BF16, tag=f"t2{tagp}", name=f"t2{tagp}_{u}")
                n = sp.tile([128, 256], BF16, tag=f"n{tagp}", name=f"n{tagp}_{u}")
                d = sp.tile([128, 256], BF16, tag=f"d{tagp}", name=f"d{tagp}_{u}")
                e = sp.tile([128, 256], BF16, tag=f"e{tagp}", name=f"e{tagp}_{u}")
                SIG = mybir.ActivationFunctionType.Sigmoid
                TANH = mybir.ActivationFunctionType.Tanh
                MUL = mybir.AluOpType.mult
                ADD = mybir.AluOpType.add
                H0 = slice(0, 128)
                H1 = slice(128, 256)
                nc.scalar.activation(r[:, H0], Prz[:, 0:128], SIG)
                nc.vector.tensor_mul(t1[:, H0], r[:, H0], Phn[:, H0])
                nc.scalar.activation(r[:, H1], Prz[:, 128:256], SIG)
                nc.vector.tensor_add(t2[:, H0], t1[:, H0], Pinn[:, H0])
                nc.scalar.activation(n[:, H0], t2[:, H0], TANH)
                nc.vector.tensor_mul(t1[:, H1], r[:, H1], Phn[:, H1])
                nc.vector.tensor_add(t2[:, H1], t1[:, H1], Pinn[:, H1])
                nc.scalar.activation(zz[:, H0], Prz[:, 256:384], SIG)
                # d = h_prev - n  (fused: (n * -1) + h_prev)
                nc.vector.scalar_tensor_tensor(
                    d[:, H0], n[:, H0], -1.0, h_prev[:, H0], MUL, ADD)
                nc.scalar.activation(n[:, H1], t2[:, H1], TANH)
                nc.vector.tensor_mul(e[:, H0], zz[:, H0], d[:, H0])
                nc.scalar.activation(zz[:, H1], Prz[:, 384:512], SIG)
                nc.vector.tensor_add(h_out[:, H0], n[:, H0], e[:, H0])
                nc.vector.transpose(h_outT[:, H0], h_out[:, H0])
                nc.vector.scalar_tensor_tensor(
                    d[:, H1], n[:, H1], -1.0, h_prev[:, H1], MUL, ADD)
                nc.vector.tensor_mul(e[:, H1], zz[:, H1], d[:, H1])
                nc.vector.tensor_add(h_out[:, H1], n[:, H1], e[:, H1])
                nc.vector.transpose(h_outT[:, H1], h_out[:, H1])

            def emit_gi(u, ks):
                """L2 gi part (needs the new h1T; k<4 only needs half 0)."""
                pw = (u + 1) % 2
                for k in ks:
                    for g in range(G):
                        mm(P2rz[32 * g:32 * g + BC, :], lT(s_h1T[pw], k),
                           s_wc_rz[:, (k * G + g) * 512:(k * G + g) * 512 + 512],
                           start=False,
                           stop=(k == 7 and not emit_rz2),
                           tile_position=(0, 32 * g))
                        mm(P2inn[32 * g:32 * g + BC, :], lT(s_h1T[pw], k),
                           s_wc_inn[:, (k * G + g) * 256:(k * G + g) * 256 + 256],
                           start=(k == 0),
                           stop=(k == 7 and not emit_inn2),
                           tile_position=(0, 32 * g))
                if 7 in ks:
                    for g in range(G):
                        if emit_rz2:
                            mm(P2rz[32 * g:32 * g + BC, :], s_ident[:],
                               s_crz2[:, g * 512:g * 512 + 512],
                               start=False, stop=True,
                               tile_position=(0, 32 * g))
                        if emit_inn2:
                            mm(P2inn[32 * g:32 * g + BC, :], s_ident[:],
                               s_cinn2[:, g * 256:g * 256 + 256],
                               start=False, stop=True,
                               tile_position=(0, 32 * g))

            def emit_film_head(u):
                """FiLM (y.T = scale.T * h2.T, per half) + head chunks k<4."""
                pw = (u + 1) % 2
                Pz = Pzs[u % 2]
                yT = sp.tile([128, 256], BF16, tag="yT", name=f"yT_{u}")
                yT_of[u] = yT
                nc.vector.tensor_mul(yT[:, 0:128], s_scaleT[:, 0:128],
                                     s_h2T[pw][:, 0:128])
                nc.vector.tensor_mul(yT[:, 128:256], s_scaleT[:, 128:256],
                                     s_h2T[pw][:, 128:256])
                for k in range(4):
                    for g in range(G):
                        mm(Pz[32 * g:32 * g + BC, :], lT(yT, k),
                           s_wh[:, (k * G + g) * LG:(k * G + g) * LG + LG],
                           start=(k == 0), stop=False, tile_position=(0, 32 * g))

            def emit_body():
                # Fully rotated software pipeline. The in-order PE stream for
                # step u is [L1z, L2gh k2-7, L2gi k0-3, L1ident(u+1),
                # L1h k0-3 (u+1), L2gi k4-7, L1h k4-7 (u+1), head k0-3,
                # head tail, L2gh k0-1 (u+1)] so every elementwise/transpose
                # wait is covered by already-ready matmul work. Step-(u+1)
                # pieces at u == U-1 wrap to the next iteration (the prologue
                # before the loop covers iteration 0).
                for u in range(U):
                    p, pw = u % 2, (u + 1) % 2
                    emit_z(u)
                    emit_gh(u, range(2, 8))
                    gru_elementwise(u, P1rz, P1inn, P1hn, s_h1s[p],
                                    s_h1s[pw], s_h1T[pw], "1")
                    emit_gi(u, range(0, 4))
                    emit_a_ident((u + 1) % U)
                    emit_a_h((u + 1) % U, range(0, 4))
                    emit_gi(u, range(4, 8))
                    gru_elementwise(u, P2rz, P2inn, P2hn, s_h2s[p],
                                    s_h2s[pw], s_h2T[pw], "2")
                    emit_a_h((u + 1) % U, range(4, 8))
                    emit_film_head(u)
                    emit_tail(u)
                    emit_gh((u + 1) % U, range(0, 2))

            emit_a_ident(0)
            emit_a_h(0, range(0, 8))
            emit_gh(0, range(0, 2))
            if use_hw_loop:
                with tc.For_i(0, T // U, 1,
                              hint_engines=(mybir.EngineType.PE,)) as it:
                    emit_body()
                    nc.sync.dma_start(d_out[:, bass.ts(it, U * LG)], s_ring[:])
            else:
                for it in range(T // U):
                    emit_body()
                    nc.sync.dma_start(
                        d_out[:, it * U * LG:(it + 1) * U * LG], s_ring[:])

    nc.compile()
    return nc


def kernel(z_start, cond_emb, max_len,
           z2h_w1, z2h_b1, z2h_w2, z2h_b2,
           w_ih1, w_hh1, b_ih1, b_hh1,
           w_ih2, w_hh2, b_ih2, b_hh2,
           film_w, film_b, head_w, head_b):
    z_start = np.asarray(z_start, np.float32)
    cond_emb = np.asarray(cond_emb, np.float32)
    T = int(max_len)
    assert T % U == 0
    f32 = lambda x: np.asarray(x, np.float32)
    w_ih1, w_hh1, b_ih1, b_hh1 = map(f32, (w_ih1, w_hh1, b_ih1, b_hh1))
    w_ih2, w_hh2, b_ih2, b_hh2 = map(f32, (w_ih2, w_hh2, b_ih2, b_hh2))
    film_w, film_b, head_w, head_b = map(f32, (film_w, film_b, head_w, head_b))
    z2h_w1, z2h_b1, z2h_w2, z2h_b2 = map(f32, (z2h_w1, z2h_b1, z2h_w2, z2h_b2))

    # ---------- host-side precompute ----------
    h0 = np.maximum(z_start @ z2h_w1.T + z2h_b1, 0.0) @ z2h_w2.T + z2h_b2
    film = cond_emb @ film_w.T + film_b
    gamma, beta = film[:, :H], film[:, H:]
    scale = 1.0 + gamma                      # [B, H]
    cz_full = beta @ head_w.T + head_b       # [B, L]
    gcond = cond_emb @ w_ih1[:, L:].T        # [B, 3H]
    crz1_full = gcond[:, :2 * H] + b_ih1[:2 * H] + b_hh1[:2 * H]
    cinn1_full = gcond[:, 2 * H:] + b_ih1[2 * H:]
    chn1_full = np.broadcast_to(b_hh1[2 * H:], (B, H)).copy()
    crz2_full = np.broadcast_to(b_ih2[:2 * H] + b_hh2[:2 * H], (B, 2 * H)).copy()
    cinn2_full = np.broadcast_to(b_ih2[2 * H:], (B, H)).copy()
    chn2_full = np.broadcast_to(b_hh2[2 * H:], (B, H)).copy()
    emit_hn1 = bool(np.any(chn1_full))
    emit_rz2 = bool(np.any(crz2_full))
    emit_inn2 = bool(np.any(cinn2_full))
    emit_hn2 = bool(np.any(chn2_full))

    # output-row permutations into the striped (group, col) layout
    perm_rz = np.concatenate([
        np.concatenate([np.arange(HG * g, HG * g + HG),
                        H + np.arange(HG * g, HG * g + HG)])
        for g in range(G)])                                   # rows of 3H
    perm_n = np.concatenate([2 * H + np.arange(HG * g, HG * g + HG)
                             for g in range(G)])
    perm_head = np.arange(L)
    cperm_rz = np.concatenate([
        np.concatenate([np.arange(HG * g, HG * g + HG),
                        H + np.arange(HG * g, HG * g + HG)])
        for g in range(G)])                                   # rows of 2H
    cperm_h = np.concatenate([np.arange(HG * g, HG * g + HG)
                              for g in range(G)])             # rows of H

    wz = w_ih1[:, :L]  # [3H, L] latent part
    wa_rz = _moving_weights(wz[perm_rz])
    wa_inn = _moving_weights(wz[perm_n])
    wb_rz = _moving_weights(w_hh1[perm_rz])
    wb_hn = _moving_weights(w_hh1[perm_n])
    wc_rz = _moving_weights(w_ih2[perm_rz])
    wc_inn = _moving_weights(w_ih2[perm_n])
    wd_rz = _moving_weights(w_hh2[perm_rz])
    wd_hn = _moving_weights(w_hh2[perm_n])
    wh = _moving_weights(head_w[perm_head])

    ident = np.eye(BC, dtype=NPBF16)

    use_hw_loop = os.environ.get("K_NO_HW_LOOP", "0") != "1"
    nc = _build_program(T, emit_hn1, emit_rz2, emit_inn2, emit_hn2,
                        use_hw_loop=use_hw_loop)

    in_maps = []
    for ci in range(NCORES):
        sl = slice(ci * BC, (ci + 1) * BC)
        m = {
            "wa_rz": wa_rz, "wa_inn": wa_inn, "wb_rz": wb_rz, "wb_hn": wb_hn,
            "wc_rz": wc_rz, "wc_inn": wc_inn, "wd_rz": wd_rz, "wd_hn": wd_hn,
            "wh": wh, "ident": ident,
            "crz1": _const_cols(crz1_full[sl], cperm_rz),
            "cinn1": _const_cols(cinn1_full[sl], cperm_h),
            "cz": _const_cols(cz_full[sl], perm_head),
            "scaleT": _scrambledT(scale[sl].astype(NPBF16)),
            "h1s0": _striped_batch(h0[sl].astype(NPBF16)),
            "h2s0": _striped_batch(h0[sl].astype(NPBF16)),
            "h1T0": _scrambledT(h0[sl].astype(NPBF16)),
            "h2T0": _scrambledT(h0[sl].astype(NPBF16)),
            "zT0": _scrambledT(z_start[sl].astype(NPBF16)),
        }
        if emit_hn1:
            m["chn1"] = _const_cols(chn1_full[sl], cperm_h)
        if emit_rz2:
            m["crz2"] = _const_cols(crz2_full[sl], cperm_rz)
        if emit_inn2:
            m["cinn2"] = _const_cols(cinn2_full[sl], cperm_h)
        if emit_hn2:
            m["chn2"] = _const_cols(chn2_full[sl], cperm_h)
        in_maps.append(m)

    trace = os.environ.get("K_TRACE", "0") == "1"
    res = bass_utils.run_bass_kernel_spmd(nc, in_maps,
                                          core_ids=list(range(NCORES)),
                                          trace=trace)
    global LAST_EXEC_NS, LAST_RESULT
    LAST_EXEC_NS = res.exec_time_ns
    LAST_RESULT = res

    out = np.empty((B, T, L), dtype=np.float32)
    for ci in range(NCORES):
        arr = res.results[ci]["out"].reshape(4, 32, T, LG)
        for g in range(G):
            out[ci * BC:(ci + 1) * BC, :, g * LG:(g + 1) * LG] = arr[g, :BC]
    return out



# revision 17
# speedup vs baseline: 1.1443x; 1.1443x over previous
"""Trainium2 Bass kernel for ConditionalLatentTrajectoryGenerator.

2-layer GRU rollout (B=128, T=512, H=1024, L=C=256) with FiLM conditioning
and an autoregressive linear head.

Sharding: data-parallel, batch 16 per core across 8 cores (weights replicated).

Per-core mapping: batch (16) is the stationary operand of every matmul
(lhsT = x.T [K,16]); weights are the moving operand, pre-permuted into 4
column-groups (tile_position col-tiling) so four weight streams run
concurrently on the PE array. Weights live in SBUF in bf16. PSUM accumulates
gi+gh for the r/z gates; per-example constants (cond-emb contribution,
biases, FiLM beta folded through the head) are added with a K=16 identity
matmul.

State h is kept striped (group g at partitions 32g..32g+16, hidden slice
[256g, 256g+256)). The x.T stationaries are refreshed each step with the
DVE 32x32 block transpose (SBUF->SBUF): out[32g+j, 32c+b] = h[b, 256g+32c+j].
The resulting block-scrambled hidden order is absorbed into the host-side
weight row permutation (moving row p of K-chunk c is hidden
S*(p//32) + 32c + p%32, S = per-stripe hidden span).
"""

import os
import sys
import numpy as np

sys.path.insert(0, "/opt/trn_rl_repo")

import ml_dtypes  # noqa: E402
from concourse import bass, bacc, mybir, tile  # noqa: E402
from concourse import bass_utils  # noqa: E402

F32 = mybir.dt.float32
BF16 = mybir.dt.bfloat16
NPBF16 = ml_dtypes.bfloat16

H = 1024
L = 256
C = 256
B = 128
NCORES = 8
BC = B // NCORES  # 16 batch per core
G = 4             # column groups / stripes
HG = H // G       # 256 hidden per group
LG = L // G       # 64 latent cols per group
U = 16            # steps per hardware-loop body
LAST_EXEC_NS = None
LAST_RESULT = None

if os.environ.get("K_LDW_OPT", "0") == "1":
    _orig_run_command = bass_utils.run_command

    def _run_command_ldwopt(cmd, **kw):
        cmd = ["--enable-ldw-opt=true" if c == "--enable-ldw-opt=false" else c
               for c in cmd]
        return _orig_run_command(cmd, **kw)

    bass_utils.run_command = _run_command_ldwopt


def _striped_batch(x):
    """[BC, 4*S] -> [128, S] with stripe g at partitions 32g..32g+BC."""
    S = x.shape[1] // G
    out = np.zeros((128, S), dtype=x.dtype)
    for g in range(G):
        out[32 * g:32 * g + BC, :] = x[:, g * S:(g + 1) * S]
    return out


def _scrambledT(x):
    """[BC, K] -> [128, K//4] block-transposed layout.

    out[32g+j, 32c+b] = x[b, S*g + 32c + j], S = K//4 — matches what
    nc.vector.transpose produces from the striped batch layout.
    """
    K = x.shape[1]
    S = K // 4
    nch = S // 32
    out = np.zeros((128, 32 * nch), dtype=x.dtype)
    for g in range(G):
        for c in range(nch):
            blk = x[:, S * g + 32 * c:S * g + 32 * c + 32]  # [BC, 32]
            out[32 * g:32 * g + 32, 32 * c:32 * c + BC][:blk.shape[1], :] = blk.T
    return out


def _k_index(K):
    """kidx[c, p] = hidden index feeding moving-row p of K-chunk c."""
    S = K // 4
    nch = S // 32
    p = np.arange(128)
    return np.stack([S * (p // 32) + 32 * c + (p % 32) for c in range(nch)])


def _moving_weights(w):
    """w [rows, K] (rows already output-permuted) -> [128, nch*G*ncols] bf16.

    Column (c, g, j) at c*G*ncols + g*ncols + j holds w[g*ncols+j, kidx[c, p]]
    for partition p.
    """
    K = w.shape[1]
    kidx = _k_index(K)                      # [nch, 128]
    ncols = w.shape[0] // G
    sel = w.T[kidx]                         # [nch, 128, G*ncols]
    arr = sel.transpose(1, 0, 2).reshape(128, kidx.shape[0] * G * ncols)
    return np.ascontiguousarray(arr.astype(NPBF16))


def _const_cols(c, perm):
    return np.ascontiguousarray(c[:, perm].astype(NPBF16))


def _build_program(T, emit_hn1, emit_rz2, emit_inn2, emit_hn2,
                   use_hw_loop=True):
    nc = bacc.Bacc("TRN2", target_bir_lowering=False, debug=False,
                   num_devices=NCORES)

    def din(name, shape, dt):
        return nc.dram_tensor(name, list(shape), dt, kind="ExternalInput")

    d_wa_rz = din("wa_rz", [128, 2 * G * 512], BF16)
    d_wa_inn = din("wa_inn", [128, 2 * G * 256], BF16)
    d_wb_rz = din("wb_rz", [128, 8 * G * 512], BF16)
    d_wb_hn = din("wb_hn", [128, 8 * G * 256], BF16)
    d_wc_rz = din("wc_rz", [128, 8 * G * 512], BF16)
    d_wc_inn = din("wc_inn", [128, 8 * G * 256], BF16)
    d_wd_rz = din("wd_rz", [128, 8 * G * 512], BF16)
    d_wd_hn = din("wd_hn", [128, 8 * G * 256], BF16)
    d_wh = din("wh", [128, 8 * G * LG], BF16)
    d_crz1 = din("crz1", [BC, G * 512], BF16)
    d_cinn1 = din("cinn1", [BC, G * 256], BF16)
    d_cz = din("cz", [BC, G * LG], BF16)
    d_chn1 = din("chn1", [BC, G * 256], BF16) if emit_hn1 else None
    d_crz2 = din("crz2", [BC, G * 512], BF16) if emit_rz2 else None
    d_cinn2 = din("cinn2", [BC, G * 256], BF16) if emit_inn2 else None
    d_chn2 = din("chn2", [BC, G * 256], BF16) if emit_hn2 else None
    d_scaleT = din("scaleT", [128, 256], BF16)
    d_ident = din("ident", [BC, BC], BF16)
    d_h1s = din("h1s0", [128, HG], BF16)
    d_h2s = din("h2s0", [128, HG], BF16)
    d_h1T = din("h1T0", [128, 256], BF16)
    d_h2T = din("h2T0", [128, 256], BF16)
    d_zT = din("zT0", [128, 64], BF16)

    d_out = nc.dram_tensor("out", [128, T * LG], F32, kind="ExternalOutput")

    def sb(name, shape, dt):
        return nc.alloc_sbuf_tensor(name, list(shape), dt)

    s_wa_rz = sb("s_wa_rz", [128, 2 * G * 512], BF16)
    s_wa_inn = sb("s_wa_inn", [128, 2 * G * 256], BF16)
    s_wb_rz = sb("s_wb_rz", [128, 8 * G * 512], BF16)
    s_wb_hn = sb("s_wb_hn", [128, 8 * G * 256], BF16)
    s_wc_rz = sb("s_wc_rz", [128, 8 * G * 512], BF16)
    s_wc_inn = sb("s_wc_inn", [128, 8 * G * 256], BF16)
    s_wd_rz = sb("s_wd_rz", [128, 8 * G * 512], BF16)
    s_wd_hn = sb("s_wd_hn", [128, 8 * G * 256], BF16)
    s_wh = sb("s_wh", [128, 8 * G * LG], BF16)
    s_crz1 = sb("s_crz1", [BC, G * 512], BF16)
    s_cinn1 = sb("s_cinn1", [BC, G * 256], BF16)
    s_cz = sb("s_cz", [BC, G * LG], BF16)
    s_chn1 = sb("s_chn1", [BC, G * 256], BF16) if emit_hn1 else None
    s_crz2 = sb("s_crz2", [BC, G * 512], BF16) if emit_rz2 else None
    s_cinn2 = sb("s_cinn2", [BC, G * 256], BF16) if emit_inn2 else None
    s_chn2 = sb("s_chn2", [BC, G * 256], BF16) if emit_hn2 else None
    s_scaleT = sb("s_scaleT", [128, 256], BF16)
    s_ident = sb("s_ident", [BC, BC], BF16)
    s_h1s = [sb(f"s_h1s{i}", [128, HG], BF16) for i in range(2)]
    s_h2s = [sb(f"s_h2s{i}", [128, HG], BF16) for i in range(2)]
    s_h1T = [sb(f"s_h1T{i}", [128, 256], BF16) for i in range(2)]
    s_h2T = [sb(f"s_h2T{i}", [128, 256], BF16) for i in range(2)]
    s_zT = [sb(f"s_zT{i}", [128, 64], BF16) for i in range(2)]
    s_ring = sb("s_ring", [128, U * LG], F32)

    with tile.TileContext(nc) as tc:
        loads = [
            (s_wa_rz, d_wa_rz), (s_wa_inn, d_wa_inn), (s_wb_rz, d_wb_rz),
            (s_wb_hn, d_wb_hn), (s_wc_rz, d_wc_rz), (s_wc_inn, d_wc_inn),
            (s_wd_rz, d_wd_rz), (s_wd_hn, d_wd_hn), (s_wh, d_wh),
            (s_crz1, d_crz1), (s_cinn1, d_cinn1), (s_cz, d_cz),
            (s_scaleT, d_scaleT), (s_ident, d_ident),
            (s_h1s[0], d_h1s), (s_h2s[0], d_h2s),
            (s_h1T[0], d_h1T), (s_h2T[0], d_h2T), (s_zT[0], d_zT),
        ]
        for s_opt, d_opt in ((s_chn1, d_chn1), (s_crz2, d_crz2),
                             (s_cinn2, d_cinn2), (s_chn2, d_chn2)):
            if s_opt is not None:
                loads.append((s_opt, d_opt))
        for s_t, d_t in loads:
            nc.sync.dma_start(s_t[:], d_t.ap())

        with tc.tile_pool(name="sp", bufs=2) as sp, \
             tc.tile_pool(name="pp", bufs=1, space="PSUM") as pp:

            P1rz = pp.tile([128, 512], F32, tag="p1rz", name="p1rz")
            P1inn = pp.tile([128, 256], F32, tag="p1inn", name="p1inn")
            P1hn = pp.tile([128, 256], F32, tag="p1hn", name="p1hn")
            P2rz = pp.tile([128, 512], F32, tag="p2rz", name="p2rz")
            P2inn = pp.tile([128, 256], F32, tag="p2inn", name="p2inn")
            P2hn = pp.tile([128, 256], F32, tag="p2hn", name="p2hn")
            Pzs = [pp.tile([128, LG], F32, tag=f"pz{i}", name=f"pz{i}")
                   for i in range(2)]
            # Initialize the never-matmul-written garbage stripes once with a
            # full-partition zero matmul (only Matmult/Memset may write PSUM;
            # DVE memset/copy to PSUM fails walrus ISA checks).
            s_zmm = sp.tile([16, 512], BF16, tag="zmm", name="s_zmm")
            nc.vector.memset(s_zmm[:], 0.0)
            for ptile, w in ((P1rz, 512), (P1inn, 256), (P1hn, 256),
                             (P2rz, 512), (P2inn, 256), (P2hn, 256),
                             (Pzs[0], LG), (Pzs[1], LG)):
                nc.tensor.matmul(ptile[:, 0:w], s_zmm[:, 0:128], s_zmm[:, 0:w],
                                 start=True, stop=True, skip_group_check=True)

            yT_of = {}

            def mm(*a, **kw):
                nc.tensor.matmul(*a, skip_group_check=True, **kw)

            def lT(t, c):
                return t[:, 32 * c:32 * c + BC]

            def emit_a_ident(u):
                """L1 const openers (always ready once the previous step's
                sigmoids have read the P1 banks)."""
                for g in range(G):
                    mm(P1rz[32 * g:32 * g + BC, :], s_ident[:],
                       s_crz1[:, g * 512:g * 512 + 512],
                       start=True, stop=False, tile_position=(0, 32 * g))
                    mm(P1inn[32 * g:32 * g + BC, :], s_ident[:],
                       s_cinn1[:, g * 256:g * 256 + 256],
                       start=True, stop=False, tile_position=(0, 32 * g))

            def emit_a_h(u, ks):
                """L1 h1 recurrent part for k in ks (k<4 only needs the first
                128-col half of h1T)."""
                p = u % 2
                for k in ks:
                    for g in range(G):
                        mm(P1rz[32 * g:32 * g + BC, :], lT(s_h1T[p], k),
                           s_wb_rz[:, (k * G + g) * 512:(k * G + g) * 512 + 512],
                           start=False, stop=False, tile_position=(0, 32 * g))
                        mm(P1hn[32 * g:32 * g + BC, :], lT(s_h1T[p], k),
                           s_wb_hn[:, (k * G + g) * 256:(k * G + g) * 256 + 256],
                           start=(k == 0),
                           stop=(k == 7 and not emit_hn1),
                           tile_position=(0, 32 * g))
                if emit_hn1 and 7 in ks:
                    for g in range(G):
                        mm(P1hn[32 * g:32 * g + BC, :], s_ident[:],
                           s_chn1[:, g * 256:g * 256 + 256],
                           start=False, stop=True, tile_position=(0, 32 * g))

            def emit_tail(u):
                """Head chunks k>=4 (need yT half 1 of step u), the Pz const
                closer, and the z feedback transpose + output-ring copy.
                Emitted inside step u+1's stream (after its L1h phase) so the
                in-order PE queue does not stall on yT half 1."""
                Pz = Pzs[u % 2]
                pw = (u + 1) % 2
                yT = yT_of.pop(u)
                for k in range(4, 8):
                    for g in range(G):
                        mm(Pz[32 * g:32 * g + BC, :], lT(yT, k),
                           s_wh[:, (k * G + g) * LG:(k * G + g) * LG + LG],
                           start=False, stop=False, tile_position=(0, 32 * g))
                for g in range(G):
                    mm(Pz[32 * g:32 * g + BC, :], s_ident[:],
                       s_cz[:, g * LG:g * LG + LG],
                       start=False, stop=True, tile_position=(0, 32 * g))
                zb = sp.tile([128, LG], BF16, tag="zb", name=f"zb_{u}")
                nc.scalar.copy(zb[:], Pz[:])
                nc.vector.transpose(s_zT[pw][:], zb[:])
                nc.vector.tensor_copy(s_ring[:, u * LG:(u + 1) * LG], Pz[:])

            def emit_z(u):
                """L1 gi latent part: closes the P1rz/P1inn groups. Needs
                zT (produced at the very end of the previous step)."""
                p = u % 2
                for k in range(2):
                    for g in range(G):
                        mm(P1rz[32 * g:32 * g + BC, :], lT(s_zT[p], k),
                           s_wa_rz[:, (k * G + g) * 512:(k * G + g) * 512 + 512],
                           start=False, stop=(k == 1), tile_position=(0, 32 * g))
                        mm(P1inn[32 * g:32 * g + BC, :], lT(s_zT[p], k),
                           s_wa_inn[:, (k * G + g) * 256:(k * G + g) * 256 + 256],
                           start=False, stop=(k == 1), tile_position=(0, 32 * g))

            def emit_gh(u, ks):
                """L2 gh part (h2T from previous step). Streams while the L1
                elementwise chain runs, hiding it from the in-order PE.
                k=0..1 are emitted at the end of the previous step's stream
                to cover the Pz->zb->zT serial tail."""
                p = u % 2
                for k in ks:
                    for g in range(G):
                        mm(P2rz[32 * g:32 * g + BC, :], lT(s_h2T[p], k),
                           s_wd_rz[:, (k * G + g) * 512:(k * G + g) * 512 + 512],
                           start=(k == 0), stop=False, tile_position=(0, 32 * g))
                        mm(P2hn[32 * g:32 * g + BC, :], lT(s_h2T[p], k),
                           s_wd_hn[:, (k * G + g) * 256:(k * G + g) * 256 + 256],
                           start=(k == 0),
                           stop=(k == 7 and not emit_hn2),
                           tile_position=(0, 32 * g))
                if emit_hn2 and 7 in ks:
                    for g in range(G):
                        mm(P2hn[32 * g:32 * g + BC, :], s_ident[:],
                           s_chn2[:, g * 256:g * 256 + 256],
                           start=False, stop=True, tile_position=(0, 32 * g))

            def gru_elementwise(u, Prz, Pinn, Phn, h_prev, h_out, h_outT,
                                tagp):
                """Column-split (two 128-col halves) GRU cell update.

                Emits ACT sigmoid/tanh and DVE mul/add/STT per half so the
                two halves pipeline across the two engines, and each half's
                32x32 block transpose lands as soon as that half of h_out is
                done (feeding the k<4 / k>=4 moving chunks)."""
                r = sp.tile([128, 256], BF16, tag=f"r{tagp}", name=f"r{tagp}_{u}")
                zz = sp.tile([128, 256], BF16, tag=f"z{tagp}", name=f"z{tagp}_{u}")
                t1 = sp.tile([128, 256], BF16, tag=f"t1{tagp}", name=f"t1{tagp}_{u}")
                t2 = sp.tile([128, 256], BF16, tag=f"t2{tagp}", name=f"t2{tagp}_{u}")
                n = sp.tile([128, 256], BF16, tag=f"n{tagp}", name=f"n{tagp}_{u}")
                d = sp.tile([128, 256], BF16, tag=f"d{tagp}", name=f"d{tagp}_{u}")
                e = sp.tile([128, 256], BF16, tag=f"e{tagp}", name=f"e{tagp}_{u}")
                SIG = mybir.ActivationFunctionType.Sigmoid
                TANH = mybir.ActivationFunctionType.Tanh
                MUL = mybir.AluOpType.mult
                ADD = mybir.AluOpType.add
                H0 = slice(0, 128)
                H1 = slice(128, 256)
                nc.scalar.activation(r[:, H0], Prz[:, 0:128], SIG)
                nc.vector.tensor_mul(t1[:, H0], r[:, H0], Phn[:, H0])
                nc.scalar.activation(r[:, H1], Prz[:, 128:256], SIG)
                nc.vector.tensor_add(t2[:, H0], t1[:, H0], Pinn[:, H0])
                nc.scalar.activation(n[:, H0], t2[:, H0], TANH)
                nc.vector.tensor_mul(t1[:, H1], r[:, H1], Phn[:, H1])
                nc.vector.tensor_add(t2[:, H1], t1[:, H1], Pinn[:, H1])
                nc.scalar.activation(zz[:, H0], Prz[:, 256:384], SIG)
                # d = h_prev - n  (fused: (n * -1) + h_prev)
                nc.vector.scalar_tensor_tensor(
                    d[:, H0], n[:, H0], -1.0, h_prev[:, H0], MUL, ADD)
                nc.scalar.activation(n[:, H1], t2[:, H1], TANH)
                nc.vector.tensor_mul(e[:, H0], zz[:, H0], d[:, H0])
                nc.scalar.activation(zz[:, H1], Prz[:, 384:512], SIG)
                nc.vector.tensor_add(h_out[:, H0], n[:, H0], e[:, H0])
                nc.vector.transpose(h_outT[:, H0], h_out[:, H0])
                nc.vector.scalar_tensor_tensor(
                    d[:, H1], n[:, H1], -1.0, h_prev[:, H1], MUL, ADD)
                nc.vector.tensor_mul(e[:, H1], zz[:, H1], d[:, H1])
                nc.vector.tensor_add(h_out[:, H1], n[:, H1], e[:, H1])
                nc.vector.transpose(h_outT[:, H1], h_out[:, H1])

            def emit_gi(u, ks):
                """L2 gi part (needs the new h1T; k<4 only needs half 0)."""
                pw = (u + 1) % 2
                for k in ks:
                    for g in range(G):
                        mm(P2rz[32 * g:32 * g + BC, :], lT(s_h1T[pw], k),
                           s_wc_rz[:, (k * G + g) * 512:(k * G + g) * 512 + 512],
                           start=False,
                           stop=(k == 7 and not emit_rz2),
                           tile_position=(0, 32 * g))
                        mm(P2inn[32 * g:32 * g + BC, :], lT(s_h1T[pw], k),
                           s_wc_inn[:, (k * G + g) * 256:(k * G + g) * 256 + 256],
                           start=(k == 0),
                           stop=(k == 7 and not emit_inn2),
                           tile_position=(0, 32 * g))
                if 7 in ks:
                    for g in range(G):
                        if emit_rz2:
                            mm(P2rz[32 * g:32 * g + BC, :], s_ident[:],
                               s_crz2[:, g * 512:g * 512 + 512],
                               start=False, stop=True,
                               tile_position=(0, 32 * g))
                        if emit_inn2:
                            mm(P2inn[32 * g:32 * g + BC, :], s_ident[:],
                               s_cinn2[:, g * 256:g * 256 + 256],
                               start=False, stop=True,
                               tile_position=(0, 32 * g))

            def emit_film_head(u):
                """FiLM (y.T = scale.T * h2.T, per half) + head chunks k<4."""
                pw = (u + 1) % 2
                Pz = Pzs[u % 2]
                yT = sp.tile([128, 256], BF16, tag="yT", name=f"yT_{u}")
                yT_of[u] = yT
                nc.vector.tensor_mul(yT[:, 0:128], s_scaleT[:, 0:128],
                                     s_h2T[pw][:, 0:128])
                nc.vector.tensor_mul(yT[:, 128:256], s_scaleT[:, 128:256],
                                     s_h2T[pw][:, 128:256])
                for k in range(4):
                    for g in range(G):
                        mm(Pz[32 * g:32 * g + BC, :], lT(yT, k),
                           s_wh[:, (k * G + g) * LG:(k * G + g) * LG + LG],
                           start=(k == 0), stop=False, tile_position=(0, 32 * g))

            def emit_body():
                # Fully rotated software pipeline. The in-order PE stream for
                # step u is [L1z, L2gh k2-7, L2gi k0-3, L1ident(u+1),
                # L1h k0-3 (u+1), L2gi k4-7, L1h k4-7 (u+1), head k0-3,
                # head tail, L2gh k0-1 (u+1)] so every elementwise/transpose
                # wait is covered by already-ready matmul work. Step-(u+1)
                # pieces at u == U-1 wrap to the next iteration (the prologue
                # before the loop covers iteration 0).
                for u in range(U):
                    p, pw = u % 2, (u + 1) % 2
                    emit_z(u)
                    emit_gh(u, range(0, 8))
                    gru_elementwise(u, P1rz, P1inn, P1hn, s_h1s[p],
                                    s_h1s[pw], s_h1T[pw], "1")
                    emit_gi(u, range(0, 8))
                    gru_elementwise(u, P2rz, P2inn, P2hn, s_h2s[p],
                                    s_h2s[pw], s_h2T[pw], "2")
                    emit_a_ident((u + 1) % U)
                    emit_a_h((u + 1) % U, range(0, 8))
                    emit_film_head(u)
                    emit_tail(u)

            emit_a_ident(0)
            emit_a_h(0, range(0, 8))
            if use_hw_loop:
                with tc.For_i(0, T // U, 1,
                              hint_engines=(mybir.EngineType.PE,)) as it:
                    emit_body()
                    nc.sync.dma_start(d_out[:, bass.ts(it, U * LG)], s_ring[:])
            else:
                for it in range(T // U):
                    emit_body()
                    nc.sync.dma_start(
                        d_out[:, it * U * LG:(it + 1) * U * LG], s_ring[:])

    nc.compile()
    return nc


def kernel(z_start, cond_emb, max_len,
           z2h_w1, z2h_b1, z2h_w2, z2h_b2,
           w_ih1, w_hh1, b_ih1, b_hh1,
           w_ih2, w_hh2, b_ih2, b_hh2,
           film_w, film_b, head_w, head_b):
    z_start = np.asarray(z_start, np.float32)
    cond_emb = np.asarray(cond_emb, np.float32)
    T = int(max_len)
    assert T % U == 0
    f32 = lambda x: np.asarray(x, np.float32)
    w_ih1, w_hh1, b_ih1, b_hh1 = map(f32, (w_ih1, w_hh1, b_ih1, b_hh1))
    w_ih2, w_hh2, b_ih2, b_hh2 = map(f32, (w_ih2, w_hh2, b_ih2, b_hh2))
    film_w, film_b, head_w, head_b = map(f32, (film_w, film_b, head_w, head_b))
    z2h_w1, z2h_b1, z2h_w2, z2h_b2 = map(f32, (z2h_w1, z2h_b1, z2h_w2, z2h_b2))

    # ---------- host-side precompute ----------
    h0 = np.maximum(z_start @ z2h_w1.T + z2h_b1, 0.0) @ z2h_w2.T + z2h_b2
    film = cond_emb @ film_w.T + film_b
    gamma, beta = film[:, :H], film[:, H:]
    scale = 1.0 + gamma                      # [B, H]
    cz_full = beta @ head_w.T + head_b       # [B, L]
    gcond = cond_emb @ w_ih1[:, L:].T        # [B, 3H]
    crz1_full = gcond[:, :2 * H] + b_ih1[:2 * H] + b_hh1[:2 * H]
    cinn1_full = gcond[:, 2 * H:] + b_ih1[2 * H:]
    chn1_full = np.broadcast_to(b_hh1[2 * H:], (B, H)).copy()
    crz2_full = np.broadcast_to(b_ih2[:2 * H] + b_hh2[:2 * H], (B, 2 * H)).copy()
    cinn2_full = np.broadcast_to(b_ih2[2 * H:], (B, H)).copy()
    chn2_full = np.broadcast_to(b_hh2[2 * H:], (B, H)).copy()
    emit_hn1 = bool(np.any(chn1_full))
    emit_rz2 = bool(np.any(crz2_full))
    emit_inn2 = bool(np.any(cinn2_full))
    emit_hn2 = bool(np.any(chn2_full))

    # output-row permutations into the striped (group, col) layout
    perm_rz = np.concatenate([
        np.concatenate([np.arange(HG * g, HG * g + HG),
                        H + np.arange(HG * g, HG * g + HG)])
        for g in range(G)])                                   # rows of 3H
    perm_n = np.concatenate([2 * H + np.arange(HG * g, HG * g + HG)
                             for g in range(G)])
    perm_head = np.arange(L)
    cperm_rz = np.concatenate([
        np.concatenate([np.arange(HG * g, HG * g + HG),
                        H + np.arange(HG * g, HG * g + HG)])
        for g in range(G)])                                   # rows of 2H
    cperm_h = np.concatenate([np.arange(HG * g, HG * g + HG)
                              for g in range(G)])             # rows of H

    wz = w_ih1[:, :L]  # [3H, L] latent part
    wa_rz = _moving_weights(wz[perm_rz])
    wa_inn = _moving_weights(wz[perm_n])
    wb_rz = _moving_weights(w_hh1[perm_rz])
    wb_hn = _moving_weights(w_hh1[perm_n])
    wc_rz = _moving_weights(w_ih2[perm_rz])
    wc_inn = _moving_weights(w_ih2[perm_n])
    wd_rz = _moving_weights(w_hh2[perm_rz])
    wd_hn = _moving_weights(w_hh2[perm_n])
    wh = _moving_weights(head_w[perm_head])

    ident = np.eye(BC, dtype=NPBF16)

    use_hw_loop = os.environ.get("K_NO_HW_LOOP", "0") != "1"
    nc = _build_program(T, emit_hn1, emit_rz2, emit_inn2, emit_hn2,
                        use_hw_loop=use_hw_loop)

    in_maps = []
    for ci in range(NCORES):
        sl = slice(ci * BC, (ci + 1) * BC)
        m = {
            "wa_rz": wa_rz, "wa_inn": wa_inn, "wb_rz": wb_rz, "wb_hn": wb_hn,
            "wc_rz": wc_rz, "wc_inn": wc_inn, "wd_rz": wd_rz, "wd_hn": wd_hn,
            "wh": wh, "ident": ident,
            "crz1": _const_cols(crz1_full[sl], cperm_rz),
            "cinn1": _const_cols(cinn1_full[sl], cperm_h),
            "cz": _const_cols(cz_full[sl], perm_head),
            "scaleT": _scrambledT(scale[sl].astype(NPBF16)),
            "h1s0": _striped_batch(h0[sl].astype(NPBF16)),
            "h2s0": _striped_batch(h0[sl].astype(NPBF16)),
            "h1T0": _scrambledT(h0[sl].astype(NPBF16)),
            "h2T0": _scrambledT(h0[sl].astype(NPBF16)),
            "zT0": _scrambledT(z_start[sl].astype(NPBF16)),
        }
        if emit_hn1:
            m["chn1"] = _const_cols(chn1_full[sl], cperm_h)
        if emit_rz2:
            m["crz2"] = _const_cols(crz2_full[sl], cperm_rz)
        if emit_inn2:
            m["cinn2"] = _const_cols(cinn2_full[sl], cperm_h)
        if emit_hn2:
            m["chn2"] = _const_cols(chn2_full[sl], cperm_h)
        in_maps.append(m)

    trace = os.environ.get("K_TRACE", "0") == "1"
    res = bass_utils.run_bass_kernel_spmd(nc, in_maps,
                                          core_ids=list(range(NCORES)),
                                          trace=trace)
    global LAST_EXEC_NS, LAST_RESULT
    LAST_EXEC_NS = res.exec_time_ns
    LAST_RESULT = res

    out = np.empty((B, T, L), dtype=np.float32)
    for ci in range(NCORES):
        arr = res.results[ci]["out"].reshape(4, 32, T, LG)
        for g in range(G):
            out[ci * BC:(ci + 1) * BC, :, g * LG:(g + 1) * LG] = arr[g, :BC]
    return out

